# revision 10
# baseline (speedup 1.0000x reference)
"""CIF (continuous integrate-and-fire) kernel for Trainium2, 8 NeuronCores.

Strategy
--------
The CIF scan over time only has a *scalar* recurrence: the integrate/fire
decisions and the per-step blend weights depend solely on ``alphas`` [B, T]
(256 KB).  All the heavy work involving ``hidden`` [B, T, H] (131 MB) is,
for fixed fire decisions, a linear map: every output frame j is a weighted
sum of consecutive hidden rows,

    out[b, j, :] = sum_t W[b, t, j] * hidden[b, t, :]

where W[b] is a [T, 64] sparse-banded weight matrix (each time step
contributes to at most two adjacent frames; weights are the reference's
``cur``/``remainds`` values).

So: replicate the reference's fp32 scalar scan on the host (exact same op
order -> bit-identical fire decisions), build W, then run the batched
[64, T] @ [T, H] matmul on the 8 NeuronCores — pure data parallel over the
batch dim, 4 rows per core, K-tiled over T with PSUM accumulation.

RAW BASS version (no TileContext): trace analysis showed the Tile
framework's end-of-context semaphore teardown costs ~8.2 us (288
EVENT_SEMAPHORE ops — one per allocated semaphore per engine) inside the
measured window.  Hand-rolling the synchronization needs only 4
semaphores:
  - sem_ring[0/1]: completion counters for the two HWDGE rings (sync /
    scalar); each ring's transfers complete in order, so a single
    monotonic counter per ring orders every consumer.
  - gdone: incremented by the last matmul of each hidden chunk-group;
    paces the rings (replicates the tile-pool flow control that keeps
    HWDGE queues shallow — deep queues measurably drop per-packet HBM
    rate ~15%) and gates each row's PSUM->SBUF cast.
  - cast_done: gates the output DMA triggers (issued last on the rings,
    so they can never head-of-line-block hidden streaming).
All buffers are unique SBUF allocations (~77 KB/partition of 208), so
there are no WAR hazards anywhere.
"""

import os

import numpy as np

# --- problem constants (hardcoded per spec: nn_CIF_Model_5970004541927) ---
B, T, H = 32, 2000, 512
NCORES = 8
R = B // NCORES          # batch rows per core = 4
ML = 64                  # MAX_LABELS
THRESH = np.float32(0.95)
P = 128                  # SBUF partitions
NFULL = T // P           # 15 full K-chunks
TAIL = T - NFULL * P     # 80 leftover time steps
NCHUNK = NFULL + 1       # 16
TP = NCHUNK * P          # 2048 (weights padded so chunks divide evenly)
GRP = int(os.environ.get("CIF_GRP", "4"))  # K-chunks per hidden DMA
DEPTH = int(os.environ.get("CIF_DEPTH", "9"))  # in-flight hidden group tiles

# matmul dtype on the PE: "fp16" (default), "fp32r", "fp32", or "bf16"
MM_MODE = os.environ.get("CIF_MM_MODE", "fp16")

_COMPILED = {}


def _build_weights(alphas: np.ndarray) -> np.ndarray:
    """Replicate the reference fp32 scan on alphas only.

    Returns WF [B, P, NCHUNK, ML] float32 — the lhsT tiles laid out so the
    device DMA reads one contiguous run per partition:
    WF[b, p, c, m] = weight of hidden step t = c*P + p into output frame m.

    Per time step t (exactly the reference ops, vectorized over the batch):
        dist_completion = 1 - integrate
        integrate += a_t ; fire = integrate > 0.95
        integrate -= fire
        cur = fire ? dist_completion : a_t   -> frame n   (n = fires so far)
        remainds = a_t - cur                 -> frame n+1  (only at a fire)
    """
    Bv, Tv = alphas.shape
    a = np.ascontiguousarray(alphas, dtype=np.float32)
    integrate = np.zeros(Bv, np.float32)
    nfires = np.zeros(Bv, np.int64)
    # two dump columns absorb contributions past frame ML-1
    WT = np.zeros((Bv, TP, ML + 2), np.float32)
    rows = np.arange(Bv)
    one = np.float32(1.0)
    for t in range(Tv):
        a_t = a[:, t]
        dist_completion = one - integrate
        integrate = integrate + a_t
        fire = integrate > THRESH
        integrate = np.where(fire, integrate - one, integrate)
        cur = np.where(fire, dist_completion, a_t)
        remainds = a_t - cur
        j = np.minimum(nfires, ML)
        WT[rows, t, j] = cur
        if fire.any():
            fr = rows[fire]
            j2 = np.minimum(nfires[fire] + 1, ML + 1)
            WT[fr, t, j2] = remainds[fire]
        nfires = nfires + fire
    WT = WT[:, :, :ML]                                  # [B, TP, ML]
    WF = WT.reshape(Bv, NCHUNK, P, ML).transpose(0, 2, 1, 3)  # [B, P, NCHUNK, ML]
    return np.ascontiguousarray(WF)


def _build_nc(mm_mode: str):
    """Emit the raw Bass program (identical on all 8 cores; SPMD over batch)."""
    import concourse.bacc as bacc
    import concourse.mybir as mybir

    f32 = mybir.dt.float32
    f16 = mybir.dt.float16
    in_dt = {
        "fp32": f32,
        "fp32r": mybir.dt.float32r,
        "bf16": mybir.dt.bfloat16,
        "fp16": mybir.dt.float16,
    }[mm_mode]
    out_dt = f32 if mm_mode in ("fp32", "fp32r") else f16

    nc = bacc.Bacc("TRN2", target_bir_lowering=False, debug=False)
    hidp = nc.dram_tensor("hidp", [R, P, NFULL * H], in_dt, kind="ExternalInput")
    hidt = nc.dram_tensor("hidt", [R, TAIL, H], in_dt, kind="ExternalInput")
    wt = nc.dram_tensor("wt", [R, P, NCHUNK * ML], in_dt, kind="ExternalInput")
    out = nc.dram_tensor("out", [R, ML, H], out_dt, kind="ExternalOutput")

    def _mk_groups():
        gs, pos = [], 0
        while pos < NFULL:
            gs.append(list(range(pos, min(pos + GRP, NFULL))))
            pos += GRP
        return gs

    groups = _mk_groups()
    NG = len(groups)

    # unique SBUF buffers: no reuse, no WAR hazards
    w_sb = [nc.alloc_sbuf_tensor(f"w{r}", [P, NCHUNK * ML], in_dt) for r in range(R)]
    t_sb = [nc.alloc_sbuf_tensor(f"t{r}", [TAIL, H], in_dt) for r in range(R)]
    h_sb = [
        [
            nc.alloc_sbuf_tensor(f"h{r}_{gi}", [P, len(g) * H], in_dt)
            for gi, g in enumerate(groups)
        ]
        for r in range(R)
    ]
    o_sb = [nc.alloc_sbuf_tensor(f"o{r}", [ML, H], out_dt) for r in range(R)]
    ps = [nc.alloc_psum_tensor(f"ps{r}", [ML, H], f32) for r in range(R)]

    gdone = nc.alloc_semaphore("gdone")
    cast_done = nc.alloc_semaphore("cast_done")

    rings = [nc.sync, nc.scalar]
    di = 0

    def issue(dst, src, pace=None):
        """Issue a DMA trigger on the next ring.

        Each trigger gets its own completion semaphore: the 16 HW DMA
        engines each bump it +1 when their share of the transfer lands,
        so sem >= 16 means the whole transfer is in SBUF.  (A shared
        per-ring counter does NOT work: different triggers' engine-share
        bumps interleave, so intermediate counts are unordered.)
        """
        nonlocal di
        ri = di % 2
        di += 1
        if pace is not None and pace > 0:
            rings[ri].wait_ge(gdone, pace)
        sem = nc.alloc_semaphore(f"dma{di}")
        rings[ri].dma_start(dst, src).then_inc(sem, 16)
        return sem

    # ---- input DMA triggers (hidden group k paced to depth DEPTH) ----
    w_dep, t_dep, h_dep = [], [], []
    for r in range(R):
        w_dep.append(issue(w_sb[r][:], wt[r]))
        t_dep.append(issue(t_sb[r][:], hidt[r]))
        deps = []
        for gi, g in enumerate(groups):
            k = r * NG + gi  # global hidden-group index
            deps.append(
                issue(
                    h_sb[r][gi][:],
                    hidp[r][:, g[0] * H : (g[-1] + 1) * H],
                    pace=k + 1 - DEPTH,
                )
            )
        h_dep.append(deps)

    # ---- matmul chains (PE in order); last matmul of each group bumps
    # gdone, which paces the rings and releases the row's cast ----
    pe = nc.tensor
    pe_seen = set()

    def pe_wait(sem):
        if sem not in pe_seen:
            pe.wait_ge(sem, 16)
            pe_seen.add(sem)

    for r in range(R):
        pe_wait(w_dep[r])
        for gi, g in enumerate(groups):
            pe_wait(h_dep[r][gi])
            for ci, c in enumerate(g):
                last_of_group = ci == len(g) - 1
                m = pe.matmul(
                    ps[r][:],
                    w_sb[r][:, c * ML : (c + 1) * ML],
                    h_sb[r][gi][:, ci * H : (ci + 1) * H],
                    start=(c == 0),
                    stop=(gi == NG - 1 and last_of_group),
                )
                if last_of_group:
                    m.then_inc(gdone)
            if gi == 0:
                pe_wait(t_dep[r])
                pe.matmul(
                    ps[r][:],
                    w_sb[r][0:TAIL, NFULL * ML : NCHUNK * ML],
                    t_sb[r][:],
                    start=False,
                    stop=False,
                )

    # ---- PSUM -> SBUF casts (vector engine) ----
    for r in range(R):
        nc.vector.wait_ge(gdone, NG * (r + 1))
        nc.vector.tensor_copy(o_sb[r][:], ps[r][:]).then_inc(cast_done)

    # ---- output triggers LAST on the rings ----
    out_sems = []
    for r in range(R):
        ri = di % 2
        di += 1
        rings[ri].wait_ge(cast_done, r + 1)
        sem = nc.alloc_semaphore(f"out{r}")
        rings[ri].dma_start(out[r], o_sb[r][:]).then_inc(sem, 16)
        out_sems.append((ri, sem))

    # make sure the program does not end before the output DMAs land
    for ri, sem in out_sems:
        rings[ri].wait_ge(sem, 16)

    nc.compile()
    return nc


def _get_nc(mm_mode: str):
    if mm_mode not in _COMPILED:
        _COMPILED[mm_mode] = _build_nc(mm_mode)
    return _COMPILED[mm_mode]


def kernel(hidden: np.ndarray, alphas: np.ndarray, _trace: bool = False):
    from concourse.bass_utils import run_bass_kernel_spmd

    hidden = np.asarray(hidden, dtype=np.float32)
    alphas = np.asarray(alphas, dtype=np.float32)
    assert hidden.shape == (B, T, H) and alphas.shape == (B, T)

    WF = _build_weights(alphas)  # [B, P, NCHUNK, ML] fp32

    # partition-major repack of the first NFULL*P steps:
    # hidp[b, p, c, h] = hidden[b, c*P + p, h]
    hidp = np.ascontiguousarray(
        hidden[:, : NFULL * P].reshape(B, NFULL, P, H).transpose(0, 2, 1, 3)
    )
    hidt = np.ascontiguousarray(hidden[:, NFULL * P :])

    if MM_MODE == "bf16":
        import ml_dtypes

        hidp = hidp.astype(ml_dtypes.bfloat16)
        hidt = hidt.astype(ml_dtypes.bfloat16)
        WF = WF.astype(ml_dtypes.bfloat16)
    elif MM_MODE == "fp16":
        hidp = hidp.astype(np.float16)
        hidt = hidt.astype(np.float16)
        WF = WF.astype(np.float16)

    hidp = hidp.reshape(B, P, NFULL * H)
    wt_dev = WF.reshape(B, P, NCHUNK * ML)

    nc = _get_nc(MM_MODE)
    in_maps = [
        {
            "hidp": hidp[c * R : (c + 1) * R],
            "hidt": hidt[c * R : (c + 1) * R],
            "wt": wt_dev[c * R : (c + 1) * R],
        }
        for c in range(NCORES)
    ]
    res = run_bass_kernel_spmd(nc, in_maps, list(range(NCORES)), trace=_trace)
    out = np.concatenate([res.results[c]["out"] for c in range(NCORES)], axis=0)
    out = np.ascontiguousarray(out.astype(np.float32))
    if _trace:
        return out, res
    return out


# revision 14
# speedup vs baseline: 1.1272x; 1.1272x over previous
"""CIF (continuous integrate-and-fire) kernel for Trainium2, 8 NeuronCores.

Strategy
--------
The CIF scan over time only has a *scalar* recurrence: the integrate/fire
decisions and the per-step blend weights depend solely on ``alphas`` [B, T]
(256 KB).  All the heavy work involving ``hidden`` [B, T, H] (131 MB) is,
for fixed fire decisions, a linear map: every output frame j is a weighted
sum of consecutive hidden rows,

    out[b, j, :] = sum_t W[b, t, j] * hidden[b, t, :]

where W[b] is a [T, 64] sparse-banded weight matrix (each time step
contributes to at most two adjacent frames; weights are the reference's
``cur``/``remainds`` values).

So: replicate the reference's fp32 scalar scan on the host (exact same op
order -> bit-identical fire decisions), build W, then run the batched
[64, T] @ [T, H] matmul on the 8 NeuronCores — pure data parallel over the
batch dim, 4 rows per core, K-tiled over T with PSUM accumulation.

RAW BASS version (no TileContext), hand-rolled synchronization:
  - one completion semaphore per DMA trigger (the 16 HW DMA engines
    each bump it +1 for their share; >=16 means the transfer landed);
  - rings self-pace on their own trigger completions (trigger #n waits
    for #n-DEPTH), keeping HWDGE queues shallow (deep queues measurably
    drop per-packet HBM rate ~15%) while staying decoupled from PE
    speed (the HAM clock gate makes PE speed erratic);
  - row_done: each row's stop matmul bumps it; releases the row's
    PSUM->SBUF cast on the vector engine;
  - cast_done: gates the output DMA triggers, issued last on the rings
    so they can never head-of-line-block hidden streaming.
All buffers are unique SBUF allocations (~77 KB/partition of 208), so
there are no WAR hazards anywhere.
"""

import os

import numpy as np

# --- problem constants (hardcoded per spec: nn_CIF_Model_5970004541927) ---
B, T, H = 32, 2000, 512
NCORES = 8
R = B // NCORES          # batch rows per core = 4
ML = 64                  # MAX_LABELS
THRESH = np.float32(0.95)
P = 128                  # SBUF partitions
NFULL = T // P           # 15 full K-chunks
TAIL = T - NFULL * P     # 80 leftover time steps
NCHUNK = NFULL + 1       # 16
TP = NCHUNK * P          # 2048 (weights padded so chunks divide evenly)
GRP = int(os.environ.get("CIF_GRP", "4"))  # K-chunks per hidden DMA
DEPTH = int(os.environ.get("CIF_DEPTH", "4"))  # per-ring in-flight DMA window

# matmul dtype on the PE: "fp16" (default), "fp32r", "fp32", or "bf16"
MM_MODE = os.environ.get("CIF_MM_MODE", "fp16")

_COMPILED = {}


def _build_weights(alphas: np.ndarray) -> np.ndarray:
    """Replicate the reference fp32 scan on alphas only.

    Returns WF [B, P, NCHUNK, ML] float32 — the lhsT tiles laid out so the
    device DMA reads one contiguous run per partition:
    WF[b, p, c, m] = weight of hidden step t = c*P + p into output frame m.

    Per time step t (exactly the reference ops, vectorized over the batch):
        dist_completion = 1 - integrate
        integrate += a_t ; fire = integrate > 0.95
        integrate -= fire
        cur = fire ? dist_completion : a_t   -> frame n   (n = fires so far)
        remainds = a_t - cur                 -> frame n+1  (only at a fire)
    """
    Bv, Tv = alphas.shape
    a = np.ascontiguousarray(alphas, dtype=np.float32)
    integrate = np.zeros(Bv, np.float32)
    nfires = np.zeros(Bv, np.int64)
    # two dump columns absorb contributions past frame ML-1
    WT = np.zeros((Bv, TP, ML + 2), np.float32)
    rows = np.arange(Bv)
    one = np.float32(1.0)
    for t in range(Tv):
        a_t = a[:, t]
        dist_completion = one - integrate
        integrate = integrate + a_t
        fire = integrate > THRESH
        integrate = np.where(fire, integrate - one, integrate)
        cur = np.where(fire, dist_completion, a_t)
        remainds = a_t - cur
        j = np.minimum(nfires, ML)
        WT[rows, t, j] = cur
        if fire.any():
            fr = rows[fire]
            j2 = np.minimum(nfires[fire] + 1, ML + 1)
            WT[fr, t, j2] = remainds[fire]
        nfires = nfires + fire
    WT = WT[:, :, :ML]                                  # [B, TP, ML]
    WF = WT.reshape(Bv, NCHUNK, P, ML).transpose(0, 2, 1, 3)  # [B, P, NCHUNK, ML]
    return np.ascontiguousarray(WF)


def _build_nc(mm_mode: str):
    """Emit the raw Bass program (identical on all 8 cores; SPMD over batch)."""
    import concourse.bacc as bacc
    import concourse.mybir as mybir

    f32 = mybir.dt.float32
    f16 = mybir.dt.float16
    in_dt = {
        "fp32": f32,
        "fp32r": mybir.dt.float32r,
        "bf16": mybir.dt.bfloat16,
        "fp16": mybir.dt.float16,
    }[mm_mode]
    out_dt = f32 if mm_mode in ("fp32", "fp32r") else f16

    nc = bacc.Bacc("TRN2", target_bir_lowering=False, debug=False)
    hidp = nc.dram_tensor("hidp", [R, P, NFULL * H], in_dt, kind="ExternalInput")
    hidt = nc.dram_tensor("hidt", [R, TAIL, H], in_dt, kind="ExternalInput")
    wt = nc.dram_tensor("wt", [R, P, NCHUNK * ML], in_dt, kind="ExternalInput")
    out = nc.dram_tensor("out", [R, ML, H], out_dt, kind="ExternalOutput")

    def _mk_groups():
        gs, pos = [], 0
        while pos < NFULL:
            gs.append(list(range(pos, min(pos + GRP, NFULL))))
            pos += GRP
        return gs

    groups = _mk_groups()
    NG = len(groups)

    # unique SBUF buffers: no reuse, no WAR hazards
    w_sb = [nc.alloc_sbuf_tensor(f"w{r}", [P, NCHUNK * ML], in_dt) for r in range(R)]
    t_sb = [nc.alloc_sbuf_tensor(f"t{r}", [TAIL, H], in_dt) for r in range(R)]
    h_sb = [
        [
            nc.alloc_sbuf_tensor(f"h{r}_{gi}", [P, len(g) * H], in_dt)
            for gi, g in enumerate(groups)
        ]
        for r in range(R)
    ]
    o_sb = [nc.alloc_sbuf_tensor(f"o{r}", [ML, H], out_dt) for r in range(R)]
    ps = [nc.alloc_psum_tensor(f"ps{r}", [ML, H], f32) for r in range(R)]

    row_done = nc.alloc_semaphore("row_done")
    cast_done = nc.alloc_semaphore("cast_done")

    rings = [nc.sync, nc.scalar]
    ring_hist = [[], []]
    di = 0

    def issue(dst, src):
        """Issue a DMA trigger on the next ring.

        Each trigger gets its own completion semaphore: the 16 HW DMA
        engines each bump it +1 when their share of the transfer lands,
        so sem >= 16 means the whole transfer is in SBUF.  (A shared
        per-ring counter does NOT work: different triggers' engine-share
        bumps interleave, so intermediate counts are unordered.)

        Pacing: a ring issues trigger #n only after its own trigger
        #n-DEPTH has fully landed.  This keeps the HWDGE queues at a
        fixed shallow depth (deep queues measurably drop per-packet HBM
        rate ~15%) while keeping DMA progress completely decoupled from
        PE progress (the HAM clock gate makes PE speed erratic).
        """
        nonlocal di
        ri = di % 2
        di += 1
        hist = ring_hist[ri]
        if len(hist) >= DEPTH:
            rings[ri].wait_ge(hist[-DEPTH], 16)
        sem = nc.alloc_semaphore(f"dma{di}")
        rings[ri].dma_start(dst, src).then_inc(sem, 16)
        hist.append(sem)
        return sem

    # ---- input DMA triggers ----
    w_dep, t_dep, h_dep = [], [], []
    for r in range(R):
        w_dep.append(issue(w_sb[r][:], wt[r]))
        t_dep.append(issue(t_sb[r][:], hidt[r]))
        deps = []
        for gi, g in enumerate(groups):
            deps.append(
                issue(h_sb[r][gi][:], hidp[r][:, g[0] * H : (g[-1] + 1) * H])
            )
        h_dep.append(deps)

    # ---- matmul chains (PE in order); each row's stop matmul bumps
    # row_done, which releases the row's cast ----
    pe = nc.tensor
    pe_seen = set()

    def pe_wait(sem):
        if sem not in pe_seen:
            pe.wait_ge(sem, 16)
            pe_seen.add(sem)

    for r in range(R):
        pe_wait(w_dep[r])
        for gi, g in enumerate(groups):
            pe_wait(h_dep[r][gi])
            for ci, c in enumerate(g):
                stop = gi == NG - 1 and ci == len(g) - 1
                m = pe.matmul(
                    ps[r][:],
                    w_sb[r][:, c * ML : (c + 1) * ML],
                    h_sb[r][gi][:, ci * H : (ci + 1) * H],
                    start=(c == 0),
                    stop=stop,
                )
                if stop:
                    m.then_inc(row_done)
            if gi == 0:
                pe_wait(t_dep[r])
                pe.matmul(
                    ps[r][:],
                    w_sb[r][0:TAIL, NFULL * ML : NCHUNK * ML],
                    t_sb[r][:],
                    start=False,
                    stop=False,
                )

    # ---- PSUM -> SBUF casts (vector engine) ----
    for r in range(R):
        nc.vector.wait_ge(row_done, r + 1)
        nc.vector.tensor_copy(o_sb[r][:], ps[r][:]).then_inc(cast_done)

    # ---- output triggers LAST on the rings ----
    out_sems = []
    for r in range(R):
        ri = di % 2
        di += 1
        rings[ri].wait_ge(cast_done, r + 1)
        sem = nc.alloc_semaphore(f"out{r}")
        rings[ri].dma_start(out[r], o_sb[r][:]).then_inc(sem, 16)
        out_sems.append((ri, sem))

    # make sure the program does not end before the output DMAs land
    for ri, sem in out_sems:
        rings[ri].wait_ge(sem, 16)

    nc.compile()
    return nc


def _get_nc(mm_mode: str):
    if mm_mode not in _COMPILED:
        _COMPILED[mm_mode] = _build_nc(mm_mode)
    return _COMPILED[mm_mode]


def kernel(hidden: np.ndarray, alphas: np.ndarray, _trace: bool = False):
    from concourse.bass_utils import run_bass_kernel_spmd

    hidden = np.asarray(hidden, dtype=np.float32)
    alphas = np.asarray(alphas, dtype=np.float32)
    assert hidden.shape == (B, T, H) and alphas.shape == (B, T)

    WF = _build_weights(alphas)  # [B, P, NCHUNK, ML] fp32

    # partition-major repack of the first NFULL*P steps:
    # hidp[b, p, c, h] = hidden[b, c*P + p, h]
    hidp = np.ascontiguousarray(
        hidden[:, : NFULL * P].reshape(B, NFULL, P, H).transpose(0, 2, 1, 3)
    )
    hidt = np.ascontiguousarray(hidden[:, NFULL * P :])

    if MM_MODE == "bf16":
        import ml_dtypes

        hidp = hidp.astype(ml_dtypes.bfloat16)
        hidt = hidt.astype(ml_dtypes.bfloat16)
        WF = WF.astype(ml_dtypes.bfloat16)
    elif MM_MODE == "fp16":
        hidp = hidp.astype(np.float16)
        hidt = hidt.astype(np.float16)
        WF = WF.astype(np.float16)

    hidp = hidp.reshape(B, P, NFULL * H)
    wt_dev = WF.reshape(B, P, NCHUNK * ML)

    nc = _get_nc(MM_MODE)
    in_maps = [
        {
            "hidp": hidp[c * R : (c + 1) * R],
            "hidt": hidt[c * R : (c + 1) * R],
            "wt": wt_dev[c * R : (c + 1) * R],
        }
        for c in range(NCORES)
    ]
    res = run_bass_kernel_spmd(nc, in_maps, list(range(NCORES)), trace=_trace)
    out = np.concatenate([res.results[c]["out"] for c in range(NCORES)], axis=0)
    out = np.ascontiguousarray(out.astype(np.float32))
    if _trace:
        return out, res
    return out
